# revision 16
# baseline (speedup 1.0000x reference)
"""AdaptiveAttention TRN2 kernel — data-parallel over batch on 8 NeuronCores.

Per-core shard: B_loc=32 of (spatial_image [B,196,1024], decoder_out [B,1024],
st [B,1024]); weights replicated. Outputs per core: alpha_t [32,196],
beta_t [32,1], c_hat [32,1024]; host concatenates along batch.

Per-core dataflow (bf16 TensorE compute, fp32 accumulation):
  phase 1 (per b): cast-DMA x_b -> bf16 SBUF; PE-transpose x_b -> xT;
    MM1 cnn^T[a,p] = W_cnn^T @ x^T; ACT tanh(+dec bias); MM2 zt = W_att^T @ tanh;
    copy zt into flat rows, DMA-scatter to zt_all [32,196].
  softmax: batched over the 32 rows; alpha, beta.
  phase 2 (per b): ct^T columns via matmul(lhsT=x_nat, rhs=alpha^T col);
    c_hat^T = ct^T + beta*(st^T - ct^T); PE-transpose back; DMA out.
"""

import sys

import numpy as np

for _p in ("/opt/trn_rl_repo",):
    if _p not in sys.path:
        sys.path.insert(0, _p)

import concourse.bass as bass
import concourse.mybir as mybir
import concourse.tile as tile
from concourse.masks import make_identity

dt = mybir.dt
AF = mybir.ActivationFunctionType
ALU = mybir.AluOpType
AX = mybir.AxisListType

B, P, H, A = 256, 196, 1024, 512
N_CORES = 8
BL = B // N_CORES  # 32
KC = H // 128      # 8 contraction chunks
ACH = A // 128     # 4 attention-dim chunks
P0 = 128
P1 = P - P0        # 68
GB = 2             # batch elements per x DMA group
NG = BL // GB      # 16 groups
ZC = 8             # b's per zt scatter chunk


def _split_multi_waits(nc):
    """Walrus in this toolchain accepts one semaphore wait per instruction.

    Tile's scheduler emits several on some instructions; hoist the extras onto
    same-engine no-ops placed immediately before (queue order preserves the
    semantics).
    """
    f = nc.m.functions[0]
    for blk in f.blocks:
        out = []
        changed = False
        for inst in blk.instructions:
            opname = type(inst).__name__
            si = getattr(inst, "sync_info", None)
            waits = list(si.on_wait) if (si is not None and si.on_wait) else []
            if len(waits) > 1:
                changed = True
                for w in waits[:-1]:
                    nop = mybir.InstNoOp(
                        name=nc.get_next_instruction_name(),
                        engine=inst.engine,
                        sync_info=mybir.SyncInfo(on_wait=[w], on_update=[]),
                        bass_nofuse=True,
                        text_hint="wait_split",
                    )
                    out.append(nop)
                inst.sync_info = mybir.SyncInfo(
                    on_wait=[waits[-1]], on_update=list(si.on_update or [])
                )
            out.append(inst)
        if changed:
            blk.instructions = out


def build(split_waits=True):
    nc = bass.Bass()
    x_d = nc.declare_dram_parameter("spatial_image", [BL, P, H], dt.float32, isOutput=False)
    dec_d = nc.declare_dram_parameter("decoder_out", [BL, H], dt.float32, isOutput=False)
    st_d = nc.declare_dram_parameter("st", [BL, H], dt.float32, isOutput=False)
    wcnn_d = nc.declare_dram_parameter("W_cnn", [H, A], dt.float32, isOutput=False)
    wdec_d = nc.declare_dram_parameter("W_dec", [H, A], dt.float32, isOutput=False)
    wsen_d = nc.declare_dram_parameter("W_sen", [H, A], dt.float32, isOutput=False)
    watt_d = nc.declare_dram_parameter("W_att", [A, 1], dt.float32, isOutput=False)
    alpha_d = nc.declare_dram_parameter("alpha_t", [BL, P], dt.float32, isOutput=True)
    beta_d = nc.declare_dram_parameter("beta_t", [BL, 1], dt.float32, isOutput=True)
    chat_d = nc.declare_dram_parameter("c_hat", [BL, H], dt.float32, isOutput=True)

    with tile.TileContext(nc) as tc:
        with (
            tc.tile_pool(name="const", bufs=1) as cp,
            tc.tile_pool(name="xnat", bufs=NG) as xp,
            tc.tile_pool(name="work", bufs=2) as wp,
        ):
            # ---------------- constants / weights ----------------
            ident_bf = cp.tile([128, 128], dt.bfloat16, name="ident_bf")
            make_identity(nc, ident_bf)
            ident_f = cp.tile([128, 128], dt.float32, name="ident_f")
            make_identity(nc, ident_f)
            ones_bf = cp.tile([1, 128], dt.bfloat16, name="ones_bf")
            nc.vector.memset(ones_bf[:], 1.0)

            wcnn_bf = cp.tile([128, KC, A], dt.bfloat16, name="wcnn_bf")
            nc.gpsimd.dma_start(
                out=wcnn_bf[:], in_=wcnn_d.rearrange("(kc p) a -> p kc a", p=128)
            )
            watt_bf = cp.tile([128, ACH], dt.bfloat16, name="watt_bf")
            nc.gpsimd.dma_start(
                out=watt_bf[:], in_=watt_d.rearrange("(c p) one -> p (c one)", p=128)
            )

            # Absorb the gpsimd identity ticks on PE before any data-dependent
            # transpose: self-loading PE matmuls tolerate only one sem wait.
            abi_f = None  # placeholder for ordering clarity
            dec_sb = cp.tile([BL, H], dt.float32, name="dec_sb")
            nc.sync.dma_start(out=dec_sb[:], in_=dec_d[:, :])
            st_sb = cp.tile([BL, H], dt.float32, name="st_sb")
            nc.sync.dma_start(out=st_sb[:], in_=st_d[:, :])

            # transposed inputs for the small matmuls + combine
            decT_bf = cp.tile([128, KC, BL], dt.bfloat16, name="decT_bf")
            stT_bf = cp.tile([128, KC, BL], dt.bfloat16, name="stT_bf")
            stT_f = cp.tile([128, KC, BL], dt.float32, name="stT_f")

            ps_cm = tc.tile_pool(name="ps", bufs=2, space="PSUM")
            ps = ps_cm.__enter__()
            if True:
                abi_f = ps.tile([1, 128], dt.float32, tag="absorb", bufs=1, name="abi_f")
                nc.tensor.transpose(abi_f, ident_f[:, 0:1], ident_f)
                abi_b = ps.tile([1, 128], dt.bfloat16, tag="absorb", bufs=1, name="abi_b")
                nc.tensor.transpose(abi_b, ident_bf[:, 0:1], ident_bf)
                for ho in range(KC):
                    pd = ps.tile([128, BL], dt.float32, tag="bankB", name="pd")
                    nc.tensor.transpose(
                        pd, dec_sb[:, ho * 128:(ho + 1) * 128], ident_f[:BL, :BL]
                    )
                    nc.vector.tensor_copy(out=decT_bf[:, ho, :], in_=pd)
                    pst = ps.tile([128, BL], dt.float32, tag="bankB", name="pst")
                    nc.tensor.transpose(
                        pst, st_sb[:, ho * 128:(ho + 1) * 128], ident_f[:BL, :BL]
                    )
                    nc.vector.tensor_copy(out=stT_bf[:, ho, :], in_=pst)
                    nc.vector.tensor_copy(out=stT_f[:, ho, :], in_=pst)

            # dec_out^T[a, b] = (decoder_out @ W_dec)^T, per 128-chunk of a
            decTo_f = cp.tile([128, ACH, BL], dt.float32, name="decTo_f")
            senT_bf = cp.tile([128, ACH, BL], dt.bfloat16, name="senT_bf")
            if True:
                for which in ("dec", "sen"):
                    wsrc = wdec_d if which == "dec" else wsen_d
                    rhsT = decT_bf if which == "dec" else stT_bf
                    pmm = [
                        ps.tile(
                            [128, BL], dt.float32, tag="bankA", bufs=4,
                            name=f"pmm{ac}",
                        )
                        for ac in range(ACH)
                    ]
                    for k in range(KC):
                        wtmp = wp.tile([128, A], dt.bfloat16, tag="wtmp", bufs=16, name="wtmp")
                        nc.gpsimd.dma_start(
                            out=wtmp[:], in_=wsrc[k * 128:(k + 1) * 128, :]
                        )
                        for ac in range(ACH):
                            nc.tensor.matmul(
                                pmm[ac],
                                wtmp[:, ac * 128:(ac + 1) * 128],
                                rhsT[:, k, :],
                                start=(k == 0),
                                stop=(k == KC - 1),
                            )
                    if which == "dec":
                        for ac in range(ACH):
                            nc.vector.tensor_copy(out=decTo_f[:, ac, :], in_=pmm[ac])
                    else:
                        # sen^T = tanh(dec_out^T + (st @ W_sen)^T)
                        for ac in range(ACH):
                            nc.vector.tensor_tensor(
                                out=pmm[ac], in0=pmm[ac], in1=decTo_f[:, ac, :],
                                op=ALU.add,
                            )
                            nc.scalar.activation(senT_bf[:, ac, :], pmm[ac], AF.Tanh)
                # out = sen @ W_att  -> [BL, 1]
                pout = ps.tile([BL, 1], dt.float32, tag="bankC", bufs=1, name="pout")
                for ac in range(ACH):
                    nc.tensor.matmul(
                        pout,
                        senT_bf[:, ac, :],
                        watt_bf[:, ac:ac + 1],
                        start=(ac == 0),
                        stop=(ac == ACH - 1),
                    )
                out_col = cp.tile([BL, 1], dt.float32, name="out_col")
                nc.vector.tensor_copy(out=out_col[:], in_=pout)

            # ---------------- phase 1: big matmul + zt ----------------
            zt_all = cp.tile([BL, P], dt.float32, name="zt_all")
            xn0g = []
            xn1g = []

            def load_group(g):
                x0 = xp.tile([128, GB, H], dt.bfloat16, tag="x0", name=f"x0_{g}")
                nc.gpsimd.dma_start(
                    out=x0[:],
                    in_=x_d[g * GB:(g + 1) * GB, 0:P0, :].rearrange("b p h -> p b h"),
                )
                x1 = xp.tile([P1, GB, H], dt.bfloat16, tag="x1", name=f"x1_{g}")
                nc.gpsimd.dma_start(
                    out=x1[:],
                    in_=x_d[g * GB:(g + 1) * GB, P0:P, :].rearrange("b p h -> p b h"),
                )
                xn0g.append(x0)
                xn1g.append(x1)
                # Dead-end "absorber" transposes: first PE touch of each new x
                # DMA lands on an instruction with no PSUM slot-reuse waits, so
                # real transposes stay within walrus's 2-sync-wait LDW budget.
                ab0 = ps.tile([1, 128], dt.bfloat16, tag="absorb", bufs=1, name="ab0")
                nc.tensor.transpose(ab0, x0[:, 0, 0:1], ident_bf)
                ab1 = ps.tile([1, P1], dt.bfloat16, tag="absorb", bufs=1, name="ab1")
                nc.tensor.transpose(ab1, x1[:, 0, 0:1], ident_bf[:P1, :P1])

            PREFETCH = 2
            for g in range(PREFETCH):
                load_group(g)

            if True:
                ztf = None
                for b in range(BL):
                    g, j = divmod(b, GB)
                    if j == 0 and g + PREFETCH < NG:
                        load_group(g + PREFETCH)
                    xn0 = xn0g[g]
                    xn1 = xn1g[g]
                    xts = []
                    for ho in range(KC):
                        pt = ps.tile([128, P], dt.bfloat16, tag="bankB", bufs=2, name="pt")
                        nc.tensor.transpose(
                            pt[:, 0:P0], xn0[:, j, ho * 128:(ho + 1) * 128], ident_bf
                        )
                        nc.tensor.transpose(
                            pt[:, P0:P], xn1[:, j, ho * 128:(ho + 1) * 128],
                            ident_bf[:P1, :P1],
                        )
                        xt = wp.tile([128, P], dt.bfloat16, tag="xt", bufs=16, name="xt")
                        nc.vector.tensor_copy(out=xt, in_=pt)
                        xts.append(xt)

                    pz = ps.tile([1, P], dt.float32, tag="bankC", bufs=1, name="pz")
                    for ac in range(ACH):
                        pc = ps.tile([128, P], dt.float32, tag="bankA", bufs=4, name="pc")
                        for k in range(KC):
                            nc.tensor.matmul(
                                pc,
                                wcnn_bf[:, k, ac * 128:(ac + 1) * 128],
                                xts[k],
                                start=(k == 0),
                                stop=(k == KC - 1),
                            )
                        th = wp.tile([128, P], dt.bfloat16, tag="th", bufs=8, name="th")
                        nc.scalar.activation(
                            th, pc, AF.Tanh, bias=decTo_f[:, ac, b:b + 1]
                        )
                        nc.tensor.matmul(
                            pz,
                            watt_bf[:, ac:ac + 1],
                            th,
                            start=(ac == 0),
                            stop=(ac == ACH - 1),
                        )
                    c, pos = divmod(b, ZC)
                    if pos == 0:
                        ztf = wp.tile([1, ZC * P], dt.float32, tag="ztf", bufs=2, name="ztf")
                    nc.vector.tensor_copy(out=ztf[0:1, pos * P:(pos + 1) * P], in_=pz)
                    if pos == ZC - 1:
                        nc.sync.dma_start(
                            out=zt_all[c * ZC:(c + 1) * ZC, :], in_=ztf[0:1, :]
                        )

            # ---------------- softmax / beta ----------------
            if True:
                nm = cp.tile([BL, 1], dt.float32, name="nm")
                nc.vector.tensor_reduce(
                    out=nm, in_=zt_all, axis=AX.X, op=ALU.max, negate=True
                )
                E_sb = cp.tile([BL, P], dt.float32, name="E_sb")
                S_sb = cp.tile([BL, 1], dt.float32, name="S_sb")
                nc.scalar.activation(E_sb, zt_all, AF.Exp, bias=nm, accum_out=S_sb)
                rS = cp.tile([BL, 1], dt.float32, name="rS")
                nc.vector.reciprocal(rS, S_sb)
                alpha_sb = cp.tile([BL, P], dt.float32, name="alpha_sb")
                nc.vector.tensor_scalar_mul(alpha_sb, E_sb, rS)
                nc.sync.dma_start(out=alpha_d[:, :], in_=alpha_sb[:])

                # beta = exp(out - m2) / (S * exp(m - m2) + exp(out - m2))
                mz = cp.tile([BL, 1], dt.float32, name="mz")
                nc.vector.tensor_scalar_mul(mz, nm, -1.0)
                m2 = cp.tile([BL, 1], dt.float32, name="m2")
                nc.vector.tensor_tensor(out=m2, in0=mz, in1=out_col, op=ALU.max)
                d1 = cp.tile([BL, 1], dt.float32, name="d1")
                nc.vector.tensor_tensor(out=d1, in0=mz, in1=m2, op=ALU.subtract)
                e1 = cp.tile([BL, 1], dt.float32, name="e1")
                nc.scalar.activation(e1, d1, AF.Exp)
                ssc = cp.tile([BL, 1], dt.float32, name="ssc")
                nc.vector.tensor_tensor(out=ssc, in0=S_sb, in1=e1, op=ALU.mult)
                d2 = cp.tile([BL, 1], dt.float32, name="d2")
                nc.vector.tensor_tensor(out=d2, in0=out_col, in1=m2, op=ALU.subtract)
                e2 = cp.tile([BL, 1], dt.float32, name="e2")
                nc.scalar.activation(e2, d2, AF.Exp)
                den = cp.tile([BL, 1], dt.float32, name="den")
                nc.vector.tensor_tensor(out=den, in0=ssc, in1=e2, op=ALU.add)
                rden = cp.tile([BL, 1], dt.float32, name="rden")
                nc.vector.reciprocal(rden, den)
                beta_sb = cp.tile([BL, 1], dt.float32, name="beta_sb")
                nc.vector.tensor_tensor(out=beta_sb, in0=e2, in1=rden, op=ALU.mult)
                nc.sync.dma_start(out=beta_d[:, :], in_=beta_sb[:])

                # alpha^T (for ct matmuls) and beta row (for the combine)
                pa0 = ps.tile([128, BL], dt.float32, tag="bankB", name="pa0")
                nc.tensor.transpose(pa0, alpha_sb[:, 0:P0], ident_f[:BL, :BL])
                aT0 = cp.tile([128, BL], dt.bfloat16, name="aT0")
                nc.vector.tensor_copy(out=aT0[:], in_=pa0)
                pa1 = ps.tile([P1, BL], dt.float32, tag="bankB", name="pa1")
                nc.tensor.transpose(pa1, alpha_sb[:, P0:P], ident_f[:BL, :BL])
                aT1 = cp.tile([P1, BL], dt.bfloat16, name="aT1")
                nc.vector.tensor_copy(out=aT1[:], in_=pa1)

                pb = ps.tile([1, BL], dt.float32, tag="bankB", name="pb")
                nc.tensor.transpose(pb, beta_sb, ident_f[:BL, :BL])
                brow8 = cp.tile([1, KC * BL], dt.bfloat16, name="brow8")
                for ho in range(KC):
                    nc.vector.tensor_copy(out=brow8[0:1, ho * BL:(ho + 1) * BL], in_=pb)

                # ---------------- phase 2: ct + c_hat ----------------
                ctT_f = cp.tile([128, KC, BL], dt.float32, name="ctT_f")
                for b in range(BL):
                    g, j = divmod(b, GB)
                    xn0 = xn0g[g]
                    xn1 = xn1g[g]
                    pct = ps.tile([128, KC], dt.float32, tag="bankA", bufs=4, name="pct")
                    for ho in range(KC):
                        nc.tensor.matmul(
                            pct[:, ho:ho + 1],
                            xn0[:, j, ho * 128:(ho + 1) * 128],
                            aT0[:, b:b + 1],
                            start=True,
                            stop=False,
                        )
                        nc.tensor.matmul(
                            pct[:, ho:ho + 1],
                            xn1[:, j, ho * 128:(ho + 1) * 128],
                            aT1[:, b:b + 1],
                            start=False,
                            stop=True,
                        )
                    nc.vector.tensor_copy(out=ctT_f[:, :, b], in_=pct)

                # beta replicated across partitions via rank-1 matmul
                pbr = ps.tile([128, KC * BL], dt.float32, tag="bankC", bufs=1, name="pbr")
                nc.tensor.matmul(pbr, ones_bf, brow8[:], start=True, stop=True)

                chT = cp.tile([128, KC, BL], dt.float32, name="chT")
                nc.vector.tensor_tensor(
                    out=chT[:, :, :], in0=stT_f[:, :, :], in1=ctT_f[:, :, :],
                    op=ALU.subtract,
                )
                nc.vector.tensor_tensor(
                    out=chT[:, :, :],
                    in0=chT[:, :, :],
                    in1=pbr.rearrange("p (kc b) -> p kc b", kc=KC),
                    op=ALU.mult,
                )
                nc.vector.tensor_tensor(
                    out=chT[:, :, :], in0=chT[:, :, :], in1=ctT_f[:, :, :], op=ALU.add
                )

                chat_nat = cp.tile([BL, H], dt.float32, name="chat_nat")
                for ho in range(KC):
                    pch = ps.tile([BL, 128], dt.float32, tag="bankB", bufs=2, name="pch")
                    nc.tensor.transpose(pch, chT[:, ho, :], ident_f)
                    nc.vector.tensor_copy(
                        out=chat_nat[:, ho * 128:(ho + 1) * 128], in_=pch
                    )
                nc.sync.dma_start(out=chat_d[:, :], in_=chat_nat[:])
            ps_cm.__exit__(None, None, None)

    if split_waits:
        _split_multi_waits(nc)
    return nc


_NC_CACHE = {}


def _get_nc():
    if "nc" not in _NC_CACHE:
        _NC_CACHE["nc"] = build()
    return _NC_CACHE["nc"]


def kernel(spatial_image, decoder_out, st, W_cnn, W_dec, W_sen, W_att):
    from concourse.bass_utils import run_bass_kernel_spmd

    nc = _get_nc()
    spatial_image = np.asarray(spatial_image, dtype=np.float32)
    decoder_out = np.asarray(decoder_out, dtype=np.float32)
    st = np.asarray(st, dtype=np.float32)
    W_cnn = np.ascontiguousarray(np.asarray(W_cnn, dtype=np.float32))
    W_dec = np.ascontiguousarray(np.asarray(W_dec, dtype=np.float32))
    W_sen = np.ascontiguousarray(np.asarray(W_sen, dtype=np.float32))
    W_att = np.ascontiguousarray(np.asarray(W_att, dtype=np.float32))

    in_maps = []
    for c in range(N_CORES):
        sl = slice(c * BL, (c + 1) * BL)
        in_maps.append(
            {
                "spatial_image": np.ascontiguousarray(spatial_image[sl]),
                "decoder_out": np.ascontiguousarray(decoder_out[sl]),
                "st": np.ascontiguousarray(st[sl]),
                "W_cnn": W_cnn,
                "W_dec": W_dec,
                "W_sen": W_sen,
                "W_att": W_att,
            }
        )
    res = run_bass_kernel_spmd(nc, in_maps, core_ids=list(range(N_CORES)))
    outs = res.results
    alpha = np.concatenate([outs[c]["alpha_t"] for c in range(N_CORES)], axis=0)
    beta = np.concatenate([outs[c]["beta_t"] for c in range(N_CORES)], axis=0)
    chat = np.concatenate([outs[c]["c_hat"] for c in range(N_CORES)], axis=0)
    return alpha, beta, chat


# revision 39
# speedup vs baseline: 565.8702x; 565.8702x over previous
"""AdaptiveAttention TRN2 kernel — data-parallel over batch on 8 NeuronCores.

Per-core shard: B_loc=32 of (spatial_image [B,196,1024], decoder_out [B,1024],
st [B,1024]); weights replicated. Outputs per core: alpha_t [32,196],
beta_t [32,1], c_hat [32,1024]; host concatenates along batch.

Per-core dataflow (bf16 TensorE compute, fp32 accumulation), pipelined in
chunks of ZC=8 batch elements so softmax/ct/c_hat overlap the main matmul:
  per b: cast-DMA x_b -> bf16 SBUF (grouped); PE-transpose x_b -> xT;
    MM1 cnn^T[a,p] = W_cnn^T @ x^T (PSUM); ACT tanh(+dec^T bias);
    MM2 zt_b = W_att^T @ tanh -> [1,196]; DVE-copy into a flat row.
  per chunk of 8 b: SBUF->SBUF DMA scatter -> zt_c [8,196]; softmax (DVE max,
    ACT exp+sum, DVE scale) -> alpha rows (DMA out); beta from (m,S,out_b);
    PE-transpose alpha rows -> alpha^T columns; per b: ct^T columns via
    matmul(lhsT=x_nat block, rhs=alpha^T col, N=1); c_hat^T chunk =
    ct^T + beta*(st^T - ct^T) (beta replicated via rank-1 matmul);
    PE-transpose back to rows; DMA out.
"""

import sys

import numpy as np

for _p in ("/opt/trn_rl_repo",):
    if _p not in sys.path:
        sys.path.insert(0, _p)

import concourse.bass as bass
import concourse.mybir as mybir
import concourse.tile as tile
from concourse.masks import make_identity

dt = mybir.dt
AF = mybir.ActivationFunctionType
ALU = mybir.AluOpType
AX = mybir.AxisListType

B, P, H, A = 256, 196, 1024, 512
N_CORES = 8
BL = B // N_CORES  # 32
KC = H // 128      # 8 contraction chunks
ACH = A // 128     # 4 attention-dim chunks
P0 = 128
P1 = P - P0        # 68
GB = 2             # batch elements per x DMA group
NG = BL // GB      # 16 groups
ZC = 8             # chunk size for softmax/ct pipeline
NCH = BL // ZC     # 4 chunks


def _split_multi_waits(nc):
    """Walrus in this toolchain accepts one semaphore wait per instruction.

    Tile's scheduler emits several on some instructions; hoist the extras onto
    same-engine no-ops placed immediately before (queue order preserves the
    semantics).
    """
    f = nc.m.functions[0]
    for blk in f.blocks:
        out = []
        changed = False
        for inst in blk.instructions:
            si = getattr(inst, "sync_info", None)
            waits = list(si.on_wait) if (si is not None and si.on_wait) else []
            if len(waits) > 1:
                changed = True
                for w in waits[:-1]:
                    nop = mybir.InstNoOp(
                        name=nc.get_next_instruction_name(),
                        engine=inst.engine,
                        sync_info=mybir.SyncInfo(on_wait=[w], on_update=[]),
                        bass_nofuse=True,
                        text_hint="wait_split",
                    )
                    out.append(nop)
                inst.sync_info = mybir.SyncInfo(
                    on_wait=[waits[-1]], on_update=list(si.on_update or [])
                )
            out.append(inst)
        if changed:
            blk.instructions = out


def build(split_waits=True):
    nc = bass.Bass()
    x_d = nc.declare_dram_parameter("spatial_image", [BL, P, H], dt.float32, isOutput=False)
    dec_d = nc.declare_dram_parameter("decoder_out", [BL, H], dt.float32, isOutput=False)
    st_d = nc.declare_dram_parameter("st", [BL, H], dt.float32, isOutput=False)
    wcnn_d = nc.declare_dram_parameter("W_cnn", [H, A], dt.float32, isOutput=False)
    wdec_d = nc.declare_dram_parameter("W_dec", [H, A], dt.float32, isOutput=False)
    wsen_d = nc.declare_dram_parameter("W_sen", [H, A], dt.float32, isOutput=False)
    watt_d = nc.declare_dram_parameter("W_att", [A, 1], dt.float32, isOutput=False)
    alpha_d = nc.declare_dram_parameter("alpha_t", [BL, P], dt.float32, isOutput=True)
    beta_d = nc.declare_dram_parameter("beta_t", [BL, 1], dt.float32, isOutput=True)
    chat_d = nc.declare_dram_parameter("c_hat", [BL, H], dt.float32, isOutput=True)

    with tile.TileContext(nc) as tc:
        with (
            tc.tile_pool(name="const", bufs=1) as cp,
            tc.tile_pool(name="xnat", bufs=NG) as xp,
            tc.tile_pool(name="work", bufs=2) as wp,
        ):
            # ---------------- constants / weights ----------------
            x0_first = xp.tile([128, GB, H], dt.bfloat16, tag="x0", name="x0_0")
            nc.gpsimd.dma_start(
                out=x0_first[:],
                in_=x_d[0:GB, 0:P0, :].rearrange("b p h -> p b h"),
            )
            x1_first = xp.tile([P1, GB, H], dt.bfloat16, tag="x1", name="x1_0")
            nc.gpsimd.dma_start(
                out=x1_first[:],
                in_=x_d[0:GB, P0:P, :].rearrange("b p h -> p b h"),
            )

            wcnn_bf = cp.tile([128, KC, A], dt.bfloat16, name="wcnn_bf")
            nc.gpsimd.dma_start(
                out=wcnn_bf[:], in_=wcnn_d.rearrange("(kc p) a -> p kc a", p=128)
            )

            ident_bf = cp.tile([128, 128], dt.bfloat16, name="ident_bf")
            make_identity(nc, ident_bf)
            ident_f = cp.tile([128, 128], dt.float32, name="ident_f")
            make_identity(nc, ident_f)
            ones_bf = cp.tile([1, 128], dt.bfloat16, name="ones_bf")
            nc.vector.memset(ones_bf[:], 1.0)

            watt_bf = cp.tile([128, ACH], dt.bfloat16, name="watt_bf")
            nc.gpsimd.dma_start(
                out=watt_bf[:], in_=watt_d.rearrange("(c p) one -> p (c one)", p=128)
            )

            dec_sb = cp.tile([BL, H], dt.float32, name="dec_sb")
            nc.sync.dma_start(out=dec_sb[:], in_=dec_d[:, :])
            st_sb = cp.tile([BL, H], dt.float32, name="st_sb")
            nc.sync.dma_start(out=st_sb[:], in_=st_d[:, :])

            decT_bf = cp.tile([128, KC, BL], dt.bfloat16, name="decT_bf")
            stT_bf = cp.tile([128, KC, BL], dt.bfloat16, name="stT_bf")

            xn0g = []
            xn1g = []

            ps_cm = tc.tile_pool(name="ps", bufs=2, space="PSUM")
            ps = ps_cm.__enter__()

            # Absorb the gpsimd identity ticks on PE first: self-loading PE
            # matmuls (transposes) tolerate only one semaphore wait.
            abi_f = ps.tile([1, 128], dt.float32, tag="absorb", bufs=1, name="abi_f")
            nc.tensor.transpose(abi_f, ident_f[:, 0:1], ident_f)
            abi_b = ps.tile([1, 128], dt.bfloat16, tag="absorb", bufs=1, name="abi_b")
            nc.tensor.transpose(abi_b, ident_bf[:, 0:1], ident_bf)

            def load_group(g):
                """Start the cast-DMAs for x batch group g (+ wait absorbers)."""
                x0 = xp.tile([128, GB, H], dt.bfloat16, tag="x0", name=f"x0_{g}")
                nc.gpsimd.dma_start(
                    out=x0[:],
                    in_=x_d[g * GB:(g + 1) * GB, 0:P0, :].rearrange("b p h -> p b h"),
                )
                x1 = xp.tile([P1, GB, H], dt.bfloat16, tag="x1", name=f"x1_{g}")
                nc.gpsimd.dma_start(
                    out=x1[:],
                    in_=x_d[g * GB:(g + 1) * GB, P0:P, :].rearrange("b p h -> p b h"),
                )
                xn0g.append(x0)
                xn1g.append(x1)
                ab0 = ps.tile([1, 128], dt.bfloat16, tag="absorb", bufs=1, name="ab0")
                nc.tensor.transpose(ab0, x0[:, 0, 0:1], ident_bf)
                ab1 = ps.tile([1, P1], dt.bfloat16, tag="absorb", bufs=1, name="ab1")
                nc.tensor.transpose(ab1, x1[:, 0, 0:1], ident_bf[:P1, :P1])

            # ---------------- setup: dec/sen path ----------------
            for ho in range(KC):
                pd = ps.tile([128, BL], dt.float32, tag="bankB", name="pd")
                nc.tensor.transpose(
                    pd, dec_sb[:, ho * 128:(ho + 1) * 128], ident_f[:BL, :BL]
                )
                nc.vector.tensor_copy(out=decT_bf[:, ho, :], in_=pd)
                pst = ps.tile([128, BL], dt.float32, tag="bankB", name="pst")
                nc.tensor.transpose(
                    pst, st_sb[:, ho * 128:(ho + 1) * 128], ident_f[:BL, :BL]
                )
                nc.vector.tensor_copy(out=stT_bf[:, ho, :], in_=pst)

            decTo_f = cp.tile([128, ACH, BL], dt.float32, name="decTo_f")
            senT_bf = cp.tile([128, ACH, BL], dt.bfloat16, name="senT_bf")
            out_row = cp.tile([1, BL], dt.float32, name="out_row")
            wtmps = {}

            def small_mm_dmas(which):
                wsrc = wdec_d if which == "dec" else wsen_d
                tiles = []
                for k in range(KC):
                    wtmp = wp.tile([128, A], dt.bfloat16, tag="wtmp", bufs=8,
                                   name="wtmp")
                    nc.gpsimd.dma_start(
                        out=wtmp[:], in_=wsrc[k * 128:(k + 1) * 128, :]
                    )
                    tiles.append(wtmp)
                wtmps[which] = tiles

            def small_mm_compute(which):
                rhsT = decT_bf if which == "dec" else stT_bf
                tiles = wtmps[which]
                for half in (0, 1):
                    pmm = [
                        ps.tile([128, BL], dt.float32, tag="bankA", bufs=3,
                                name=f"pmm{ac}")
                        for ac in (2 * half, 2 * half + 1)
                    ]
                    for k in range(KC):
                        for i, ac in enumerate((2 * half, 2 * half + 1)):
                            nc.tensor.matmul(
                                pmm[i],
                                tiles[k][:, ac * 128:(ac + 1) * 128],
                                rhsT[:, k, :],
                                start=(k == 0),
                                stop=(k == KC - 1),
                            )
                    for i, ac in enumerate((2 * half, 2 * half + 1)):
                        if which == "dec":
                            nc.vector.tensor_copy(
                                out=decTo_f[:, ac, :], in_=pmm[i]
                            )
                        else:
                            nc.vector.tensor_tensor(
                                out=pmm[i], in0=pmm[i], in1=decTo_f[:, ac, :],
                                op=ALU.add,
                            )
                            nc.scalar.activation(
                                senT_bf[:, ac, :], pmm[i], AF.Tanh
                            )
                if which == "sen":
                    # out = sen @ W_att -> [BL,1] column, then a [1,BL] row
                    pout = ps.tile([BL, 1], dt.float32, tag="bankB", bufs=2,
                                   name="pout")
                    for ac in range(ACH):
                        nc.tensor.matmul(
                            pout,
                            senT_bf[:, ac, :],
                            watt_bf[:, ac:ac + 1],
                            start=(ac == 0),
                            stop=(ac == ACH - 1),
                        )
                    out_col = cp.tile([BL, 1], dt.float32, name="out_col")
                    nc.vector.tensor_copy(out=out_col[:], in_=pout)
                    porow = ps.tile([1, BL], dt.float32, tag="bankB", name="porow")
                    nc.tensor.transpose(porow, out_col, ident_f[:BL, :BL])
                    nc.vector.tensor_copy(out=out_row[:], in_=porow)

            small_mm_dmas("dec")
            small_mm_compute("dec")

            # group 0 was DMA'd at the very top; emit its absorbers now and
            # register the tiles.
            xn0g.append(x0_first)
            xn1g.append(x1_first)
            ab0f = ps.tile([1, 128], dt.bfloat16, tag="absorb", bufs=1, name="ab0f")
            nc.tensor.transpose(ab0f, x0_first[:, 0, 0:1], ident_bf)
            ab1f = ps.tile([1, P1], dt.bfloat16, tag="absorb", bufs=1, name="ab1f")
            nc.tensor.transpose(ab1f, x1_first[:, 0, 0:1], ident_bf[:P1, :P1])

            PREFETCH = 3
            for g in range(1, PREFETCH):
                load_group(g)

            # chunk-persistent destinations
            aT0 = cp.tile([128, BL], dt.bfloat16, name="aT0")
            aT1 = cp.tile([P1, BL], dt.bfloat16, name="aT1")
            ct_all = cp.tile([BL, H], dt.float32, name="ct_all")
            beta_all = cp.tile([BL, 1], dt.float32, name="beta_all")

            def do_chunk(c, zt_c):
                """Softmax + beta + alpha^T + ct + c_hat for rows
                [c*ZC, (c+1)*ZC)."""
                b0 = c * ZC

                nm = wp.tile([ZC, 1], dt.float32, tag="nm", bufs=2, name="nm")
                nc.vector.tensor_reduce(
                    out=nm, in_=zt_c, axis=AX.X, op=ALU.max, negate=True
                )
                alf = wp.tile([ZC, P], dt.float32, tag="alf", bufs=2, name="alf")
                S_c = wp.tile([ZC, 1], dt.float32, tag="S_c", bufs=2, name="S_c")
                nc.scalar.activation(alf, zt_c, AF.Exp, bias=nm, accum_out=S_c)
                rS = wp.tile([ZC, 1], dt.float32, tag="rS", bufs=2, name="rS")
                nc.vector.reciprocal(rS, S_c)
                nc.vector.tensor_scalar_mul(alf, alf, rS)
                nc.sync.dma_start(out=alpha_d[b0:b0 + ZC, :], in_=alf[:, :])

                # beta = e2 / (S*e1 + e2); m2 = max(mz, out); e_i = exp(.-m2)
                po_c = ps.tile([ZC, 1], dt.float32, tag="bankB", name="po_c")
                nc.tensor.transpose(
                    po_c, out_row[0:1, b0:b0 + ZC], ident_f[:1, :1]
                )
                mz = wp.tile([ZC, 1], dt.float32, tag="mz", bufs=2, name="mz")
                nc.vector.tensor_scalar_mul(mz, nm, -1.0)
                m2 = wp.tile([ZC, 1], dt.float32, tag="m2", bufs=2, name="m2")
                nc.vector.tensor_tensor(out=m2, in0=mz, in1=po_c, op=ALU.max)
                d1 = wp.tile([ZC, 1], dt.float32, tag="d1", bufs=2, name="d1")
                nc.vector.tensor_tensor(out=d1, in0=mz, in1=m2, op=ALU.subtract)
                e1 = wp.tile([ZC, 1], dt.float32, tag="e1", bufs=2, name="e1")
                nc.scalar.activation(e1, d1, AF.Exp)
                ssc = wp.tile([ZC, 1], dt.float32, tag="ssc", bufs=2, name="ssc")
                nc.vector.tensor_tensor(out=ssc, in0=S_c, in1=e1, op=ALU.mult)
                d2 = wp.tile([ZC, 1], dt.float32, tag="d2", bufs=2, name="d2")
                nc.vector.tensor_tensor(out=d2, in0=po_c, in1=m2, op=ALU.subtract)
                e2 = wp.tile([ZC, 1], dt.float32, tag="e2", bufs=2, name="e2")
                nc.scalar.activation(e2, d2, AF.Exp)
                den = wp.tile([ZC, 1], dt.float32, tag="den", bufs=2, name="den")
                nc.vector.tensor_tensor(out=den, in0=ssc, in1=e2, op=ALU.add)
                rden = wp.tile([ZC, 1], dt.float32, tag="rden", bufs=2, name="rden")
                nc.vector.reciprocal(rden, den)
                bta = wp.tile([ZC, 1], dt.float32, tag="bta", bufs=2, name="bta")
                nc.vector.tensor_tensor(out=bta, in0=e2, in1=rden, op=ALU.mult)
                nc.sync.dma_start(out=beta_d[b0:b0 + ZC, :], in_=bta[:, :])

                # alpha^T columns for this chunk (bf16 for the ct matmuls)
                pa0 = ps.tile([128, ZC], dt.float32, tag="bankB", name="pa0")
                nc.tensor.transpose(pa0, alf[:, 0:P0], ident_f[:ZC, :ZC])
                nc.vector.tensor_copy(out=aT0[:, b0:b0 + ZC], in_=pa0)
                pa1 = ps.tile([P1, ZC], dt.float32, tag="bankB", name="pa1")
                nc.tensor.transpose(pa1, alf[:, P0:P], ident_f[:ZC, :ZC])
                nc.vector.tensor_copy(out=aT1[:, b0:b0 + ZC], in_=pa1)
                # 32x-replicated alpha^T columns (so each ct matmul fills a
                # whole 32-partition PSUM block)
                a0r = wp.tile([128, ZC, 32], dt.bfloat16, tag="a0r", bufs=2,
                              name="a0r")
                nc.vector.tensor_copy(
                    out=a0r,
                    in_=pa0.rearrange("p (z one) -> p z one", one=1).broadcast_to(
                        [128, ZC, 32]
                    ),
                )
                a1r = wp.tile([P1, ZC, 32], dt.bfloat16, tag="a1r", bufs=2,
                              name="a1r")
                nc.vector.tensor_copy(
                    out=a1r,
                    in_=pa1.rearrange("p (z one) -> p z one", one=1).broadcast_to(
                        [P1, ZC, 32]
                    ),
                )
                nc.sync.dma_start(
                    out=beta_all[b0:b0 + ZC, :], in_=bta[:, :]
                )

                # ct rows: stream x through PE with alpha^T columns as the
                # (trivial-load) stationary operand; pack 4 b per PSUM bank at
                # partition offsets 0/32/64/96, copy out in one DVE op, then
                # DMA-gather the 4 rows into ct_all.
                for q in range(ZC // 4):
                    bq = b0 + 4 * q
                    for ht in range(2):
                        pct4 = ps.tile([128, 512], dt.float32, tag="ct4",
                                       bufs=2, name="pct4")
                        for mi in range(4):
                            bb = bq + mi
                            g2, j2 = divmod(bb, GB)
                            hs = slice(ht * 512, (ht + 1) * 512)
                            nc.tensor.matmul(
                                pct4[32 * mi:32 * mi + 32, :],
                                a0r[:, bb - b0, :],
                                xn0g[g2][:, j2, hs],
                                start=True,
                                stop=False,
                                tile_position=(0, 32 * mi),
                            )
                            nc.tensor.matmul(
                                pct4[32 * mi:32 * mi + 32, :],
                                a1r[:, bb - b0, :],
                                xn1g[g2][:, j2, hs],
                                start=False,
                                stop=True,
                                tile_position=(0, 32 * mi),
                            )
                        ct4sb = wp.tile([128, 512], dt.float32, tag="ct4sb",
                                        bufs=4, name="ct4sb")
                        nc.vector.tensor_copy(out=ct4sb[:], in_=pct4[:])
                        nc.sync.dma_start(
                            out=ct_all[bq:bq + 4, ht * 512:(ht + 1) * 512],
                            in_=ct4sb.rearrange("(q r) f -> q r f", q=4)[:, 0, :],
                        )

            # ------------- main pipeline over 2-b macro tiles -------------
            P2 = 2 * P  # 392: moving dim for a b-pair
            MPC = ZC // GB  # macros per chunk (4)
            DELAY = 2  # macros of pipeline delay before chunk processing
            pending = []
            for m in range(BL // GB):
                b0_, b1_ = 2 * m, 2 * m + 1
                if m == 1:
                    small_mm_dmas("sen")
                if m == 3:
                    small_mm_compute("sen")
                if m + PREFETCH < NG:
                    load_group(m + PREFETCH)
                if pending and m >= (pending[0][0] + 1) * MPC + DELAY:
                    pc_, pzf_ = pending.pop(0)
                    do_chunk(pc_, pzf_)
                xn0 = xn0g[m]
                xn1 = xn1g[m]
                xts = []
                for ho in range(KC):
                    pt = ps.tile([128, P2], dt.bfloat16, tag="bankB", bufs=2,
                                 name="pt")
                    for j in range(GB):
                        nc.tensor.transpose(
                            pt[:, j * P:j * P + P0],
                            xn0[:, j, ho * 128:(ho + 1) * 128],
                            ident_bf,
                        )
                        nc.tensor.transpose(
                            pt[:, j * P + P0:(j + 1) * P],
                            xn1[:, j, ho * 128:(ho + 1) * 128],
                            ident_bf[:P1, :P1],
                        )
                    xt = wp.tile([128, P2], dt.bfloat16, tag="xt", bufs=16,
                                 name="xt")
                    nc.vector.tensor_copy(out=xt, in_=pt)
                    xts.append(xt)

                c, pos = divmod(m, MPC)
                if pos == 0:
                    zt_c = wp.tile([ZC, P], dt.float32, tag="zt_c", bufs=2,
                                   name="zt_c")
                pz = ps.tile([1, P2], dt.float32, tag="absorb", bufs=1, name="pz")
                for ac in range(ACH):
                    pc = ps.tile([128, P2], dt.float32, tag="bankA", bufs=3,
                                 name="pc")
                    for k in range(KC):
                        nc.tensor.matmul(
                            pc,
                            wcnn_bf[:, k, ac * 128:(ac + 1) * 128],
                            xts[k],
                            start=(k == 0),
                            stop=(k == KC - 1),
                        )
                    th = wp.tile([128, P2], dt.bfloat16, tag="th", bufs=4,
                                 name="th")
                    nc.scalar.activation(
                        th[:, 0:P], pc[:, 0:P], AF.Tanh,
                        bias=decTo_f[:, ac, b0_:b0_ + 1],
                    )
                    nc.scalar.activation(
                        th[:, P:P2], pc[:, P:P2], AF.Tanh,
                        bias=decTo_f[:, ac, b1_:b1_ + 1],
                    )
                    nc.tensor.matmul(
                        pz,
                        watt_bf[:, ac:ac + 1],
                        th,
                        start=(ac == 0),
                        stop=(ac == ACH - 1),
                    )
                zrow = wp.tile([1, P2], dt.float32, tag="zrow", bufs=2,
                               name="zrow")
                nc.vector.tensor_copy(out=zrow[:], in_=pz)
                nc.sync.dma_start(
                    out=zt_c[GB * pos:GB * (pos + 1), :], in_=zrow[0:1, :]
                )
                if pos == MPC - 1:
                    pending.append((c, zt_c))
            for pc_, pzf_ in pending:
                do_chunk(pc_, pzf_)

            # c_hat = ct + beta*(st - ct), batched; halves so the first
            # output DMA overlaps the second half's compute
            chat_sb = cp.tile([BL, H], dt.float32, name="chat_sb")
            for hh in range(2):
                hsl = slice(hh * (H // 2), (hh + 1) * (H // 2))
                nc.vector.tensor_tensor(
                    out=chat_sb[:, hsl], in0=st_sb[:, hsl], in1=ct_all[:, hsl],
                    op=ALU.subtract,
                )
                nc.vector.tensor_scalar_mul(
                    chat_sb[:, hsl], chat_sb[:, hsl], beta_all
                )
                nc.vector.tensor_tensor(
                    out=chat_sb[:, hsl], in0=chat_sb[:, hsl],
                    in1=ct_all[:, hsl], op=ALU.add,
                )
                nc.sync.dma_start(out=chat_d[:, hsl], in_=chat_sb[:, hsl])

            ps_cm.__exit__(None, None, None)

    if split_waits:
        _split_multi_waits(nc)
    return nc


_NC_CACHE = {}


def _get_nc():
    if "nc" not in _NC_CACHE:
        _NC_CACHE["nc"] = build()
    return _NC_CACHE["nc"]


def kernel(spatial_image, decoder_out, st, W_cnn, W_dec, W_sen, W_att):
    from concourse.bass_utils import run_bass_kernel_spmd

    nc = _get_nc()
    spatial_image = np.asarray(spatial_image, dtype=np.float32)
    decoder_out = np.asarray(decoder_out, dtype=np.float32)
    st = np.asarray(st, dtype=np.float32)
    W_cnn = np.ascontiguousarray(np.asarray(W_cnn, dtype=np.float32))
    W_dec = np.ascontiguousarray(np.asarray(W_dec, dtype=np.float32))
    W_sen = np.ascontiguousarray(np.asarray(W_sen, dtype=np.float32))
    W_att = np.ascontiguousarray(np.asarray(W_att, dtype=np.float32))

    in_maps = []
    for c in range(N_CORES):
        sl = slice(c * BL, (c + 1) * BL)
        in_maps.append(
            {
                "spatial_image": np.ascontiguousarray(spatial_image[sl]),
                "decoder_out": np.ascontiguousarray(decoder_out[sl]),
                "st": np.ascontiguousarray(st[sl]),
                "W_cnn": W_cnn,
                "W_dec": W_dec,
                "W_sen": W_sen,
                "W_att": W_att,
            }
        )
    res = run_bass_kernel_spmd(nc, in_maps, core_ids=list(range(N_CORES)))
    outs = res.results
    alpha = np.concatenate([outs[c]["alpha_t"] for c in range(N_CORES)], axis=0)
    beta = np.concatenate([outs[c]["beta_t"] for c in range(N_CORES)], axis=0)
    chat = np.concatenate([outs[c]["c_hat"] for c in range(N_CORES)], axis=0)
    return alpha, beta, chat
